# revision 6
# baseline (speedup 1.0000x reference)
"""Trainium2 Bass kernel for nn_Attention_80212809220238.

Data-parallel over batch (B=8 -> 1 batch element per NeuronCore).
Per core, full attention layer on [1024, 768] in transposed layouts:
  - qkT = Wqk_aug @ xT_aug   (q,k in [channels, tokens]; biases via ones-row)
  - LN over head_dim in transposed layout: per-token stats via PE selector
    matmuls; rank-1 broadcasts A = w(x)r and E = -w(x)mu*r materialized by
    K=12 selector matmuls; applied in 2 DVE passes.
  - v in natural layout [tokens, channels] + ones column -> v_aug
  - scoresT = k~^T q~ per head; exp on ACT with 1/sqrt(D) folded into scale
    (no max-subtraction: LN bounds |scores|/8 <= 8, exp safe in fp32)
  - out.T = v_aug^T @ attnT: row 64 = softmax denominator for free
  - normalize via K=1 ones broadcast matmul + DVE multiply
  - y.T = Wp_aug @ outT (bias via ones row), DMA straight to DRAM
All matmuls float32r (fp22 multiply, fp32 accumulate, full PE rate).
"""

import sys
from contextlib import ExitStack

import numpy as np

if "/opt/trn_rl_repo" not in sys.path:
    sys.path.insert(0, "/opt/trn_rl_repo")

B, N, C = 8, 1024, 768
H, D = 12, 64
EPS = 1e-5
NCORES = 8
CH = (slice(0, 512), slice(512, 1024))  # token chunks

_CACHE: dict = {}


def _build(loop_k: int = 1):
    key = ("nc", loop_k)
    if key in _CACHE:
        return _CACHE[key]
    import concourse.mybir as mybir
    import concourse.tile as tile
    from concourse import bacc

    f32, f32r = mybir.dt.float32, mybir.dt.float32r

    nc = bacc.Bacc("TRN2", target_bir_lowering=False, debug=False,
                   num_devices=NCORES)
    prm = dict(
        xT=nc.declare_dram_parameter("xT", [C + 1, N], f32r, isOutput=False),
        wqk=nc.declare_dram_parameter("wqk", [C + 1, 2 * C], f32r,
                                      isOutput=False),
        wv=nc.declare_dram_parameter("wv", [C + 1, C], f32r, isOutput=False),
        wp=nc.declare_dram_parameter("wp", [C + 1, C], f32r, isOutput=False),
        selA=nc.declare_dram_parameter("selA", [12, 12, 128], f32r,
                                       isOutput=False),
        selE=nc.declare_dram_parameter("selE", [12, 12, 128], f32r,
                                       isOutput=False),
        sumsel=nc.declare_dram_parameter("sumsel", [6, 128, 12], f32r,
                                         isOutput=False),
        lnb=nc.declare_dram_parameter("lnb", [2, 128, 1], f32, isOutput=False),
        onesr=nc.declare_dram_parameter("onesr", [1, 64], f32r,
                                        isOutput=False),
        yT=nc.declare_dram_parameter("yT", [C, N], f32, isOutput=True),
    )

    with tile.TileContext(nc) as tc:
        with nc.allow_low_precision("fp32r attention"):
            if loop_k == 1:
                _emit(nc, tc, prm)
            else:
                with tc.For_i(0, loop_k, 1) as _:
                    _emit(nc, tc, prm)
    nc.compile()
    _CACHE[key] = nc
    return nc


def _emit(nc, tc, prm):
    import concourse.mybir as mybir

    f32, f32r = mybir.dt.float32, mybir.dt.float32r
    AF = mybir.ActivationFunctionType
    Alu = mybir.AluOpType
    xT, wqk, wv, wp = prm["xT"], prm["wqk"], prm["wv"], prm["wp"]
    selA, selE, sumsel, lnb, yT = (prm["selA"], prm["selE"], prm["sumsel"],
                                   prm["lnb"], prm["yT"])
    onesr = prm["onesr"]

    wqk_r = wqk[0:C, :].rearrange("(t p) c -> p t c", p=128)
    wv_r = wv[0:C, :].rearrange("(t p) c -> p t c", p=128)
    wp_h = wp[0:C, :].rearrange("(h d) c -> d h c", d=64)

    with ExitStack() as ctx:
        consts = ctx.enter_context(tc.tile_pool(name="consts", bufs=1))
        selA_sb = consts.tile([12, 12, 128], f32r, tag="selA")
        nc.sync.dma_start(out=selA_sb[:], in_=selA.rearrange("m k p -> k m p"))
        selE_sb = consts.tile([12, 12, 128], f32r, tag="selE")
        nc.sync.dma_start(out=selE_sb[:], in_=selE.rearrange("m k p -> k m p"))
        sumsel_sb = consts.tile([128, 6, 12], f32r, tag="sumsel")
        nc.sync.dma_start(out=sumsel_sb[:],
                          in_=sumsel.rearrange("l p j -> p l j"))
        lnbq = consts.tile([128, 1], f32, tag="lnbq")
        nc.sync.dma_start(out=lnbq[:], in_=lnb[0])
        lnbk = consts.tile([128, 1], f32, tag="lnbk")
        nc.sync.dma_start(out=lnbk[:], in_=lnb[1])
        wqk_brow = consts.tile([1, 2 * C], f32r, tag="wqk_brow")
        nc.sync.dma_start(out=wqk_brow[:], in_=wqk[C:C + 1, :])
        wv_brow = consts.tile([1, C], f32r, tag="wv_brow")
        nc.sync.dma_start(out=wv_brow[:], in_=wv[C:C + 1, :])
        wp_brow = consts.tile([1, C], f32r, tag="wp_brow")
        nc.sync.dma_start(out=wp_brow[:], in_=wp[C:C + 1, :])
        ones64 = consts.tile([65, 64], f32r, tag="ones64")
        nc.sync.dma_start(out=ones64[64:65, :], in_=onesr[0:1, :])
        eps12 = consts.tile([12, 1], f32, tag="eps12")
        nc.vector.memset(eps12[:], EPS)
        xt_ones = consts.tile([1, N], f32r, tag="xt_ones")
        nc.sync.dma_start(out=xt_ones[:], in_=xT[C:C + 1, :])

        qknp = ctx.enter_context(tc.tile_pool(name="qknp", bufs=1))
        qkn = [qknp.tile([128, N], f32r, name=f"qkn{m}", tag=f"qkn{m}")
               for m in range(12)]
        vp = ctx.enter_context(tc.tile_pool(name="vp", bufs=1))
        v_sb = [vp.tile([128, H, 65], f32r, name=f"v{t}", tag=f"v{t}")
                for t in range(8)]

        with ExitStack() as xctx:
            xp = xctx.enter_context(tc.tile_pool(name="xp", bufs=1))
            xt = [xp.tile([128, N], f32r, name=f"xt{k}", tag=f"xt{k}")
                  for k in range(6)]
            for k in range(6):
                nc.sync.dma_start(out=xt[k][:],
                                  in_=xT[k * 128:(k + 1) * 128, :])

            # ---- QK + LN, one side at a time (q: m=0..5, k: m=6..11) -----
            for side in range(2):
                m0 = side * 6
                with ExitStack() as sctx:
                    rawp = sctx.enter_context(
                        tc.tile_pool(name=f"raw{side}", bufs=6))
                    stsb = sctx.enter_context(
                        tc.tile_pool(name=f"stsb{side}", bufs=1))
                    s1p = stsb.tile([12, N], f32, tag="s1p")
                    s2p = stsb.tile([12, N], f32, tag="s2p")
                    raws = []
                    with ExitStack() as mctx:
                        qkps = mctx.enter_context(tc.tile_pool(
                            name=f"qkps{side}", bufs=2, space="PSUM"))
                        stps = mctx.enter_context(tc.tile_pool(
                            name=f"stps{side}", bufs=1, space="PSUM"))
                        wsl = mctx.enter_context(
                            tc.tile_pool(name=f"wsl{side}", bufs=2))
                        sqp = mctx.enter_context(
                            tc.tile_pool(name=f"sq{side}", bufs=2))
                        stt = [stps.tile([12, 512], f32, name=f"st{side}_{j}",
                                         tag=f"st{j}") for j in range(4)]
                        for ml in range(6):
                            m = m0 + ml
                            ws = wsl.tile([128, 6, 128], f32r, tag="ws")
                            nc.sync.dma_start(
                                out=ws[:],
                                in_=wqk_r[:, :, m * 128:(m + 1) * 128])
                            ps = qkps.tile([128, N], f32, tag="qk")
                            for c in range(2):
                                for k in range(6):
                                    nc.tensor.matmul(
                                        ps[:, CH[c]], ws[:, k, :],
                                        xt[k][:, CH[c]],
                                        start=(k == 0), stop=False)
                                nc.tensor.matmul(
                                    ps[:, CH[c]],
                                    wqk_brow[0:1, m * 128:(m + 1) * 128],
                                    xt_ones[0:1, CH[c]], start=False,
                                    stop=True)
                            raw = rawp.tile([128, N], f32r, tag="raw")
                            raws.append(raw)
                            nc.scalar.activation(out=raw[:], in_=ps[:],
                                                 func=AF.Copy)
                            sq = sqp.tile([128, N], f32r, tag="sq")
                            nc.vector.tensor_mul(sq[:], raw[:], raw[:])
                            # accumulate per-token sums into packed [12, 512]
                            # rows via one-hot selector columns
                            for c in range(2):
                                nc.tensor.matmul(
                                    stt[c][:], sumsel_sb[:, ml, :],
                                    raw[:, CH[c]],
                                    start=(ml == 0), stop=(ml == 5))
                                nc.tensor.matmul(
                                    stt[2 + c][:], sumsel_sb[:, ml, :],
                                    sq[:, CH[c]],
                                    start=(ml == 0), stop=(ml == 5))
                        for c in range(2):
                            nc.vector.tensor_copy(s1p[:, CH[c]], stt[c][:])
                            nc.vector.tensor_copy(s2p[:, CH[c]], stt[2 + c][:])

                    # packed LN stats on [12, N] tiles
                    tsq = stsb.tile([12, N], f32, tag="tsq")
                    nc.vector.tensor_mul(tsq[:], s1p[:], s1p[:])
                    ve = stsb.tile([12, N], f32, tag="ve")
                    nc.vector.scalar_tensor_tensor(
                        out=ve[:], in0=tsq[:], scalar=1.0 / 64.0, in1=s2p[:],
                        op0=Alu.mult, op1=Alu.subtract)  # S1^2/64 - S2
                    std = stsb.tile([12, N], f32, tag="std")
                    nc.scalar.activation(out=std[:], in_=ve[:], func=AF.Sqrt,
                                         bias=eps12[:], scale=-1.0 / 64.0)
                    r_pk = stsb.tile([12, N], f32r, tag="r_pk")
                    nc.vector.reciprocal(r_pk[:], std[:])
                    mur = stsb.tile([12, N], f32r, tag="mur")
                    nc.vector.scalar_tensor_tensor(
                        out=mur[:], in0=s1p[:], scalar=1.0 / 64.0, in1=r_pk[:],
                        op0=Alu.mult, op1=Alu.mult)

                    # apply: qkn = raw*(w(x)r) + lnb + (-w(x)mu r)
                    with ExitStack() as actx:
                        aps = actx.enter_context(tc.tile_pool(
                            name=f"aps{side}", bufs=2, space="PSUM"))
                        eps_ = actx.enter_context(tc.tile_pool(
                            name=f"eps{side}", bufs=2, space="PSUM"))
                        t1p = actx.enter_context(
                            tc.tile_pool(name=f"t1{side}", bufs=2))
                        for ml in range(6):
                            m = m0 + ml
                            A = aps.tile([128, N], f32, tag="A")
                            E = eps_.tile([128, N], f32, tag="E")
                            for c in range(2):
                                nc.tensor.matmul(
                                    A[:, CH[c]], selA_sb[:, m, :],
                                    r_pk[:, CH[c]], start=True, stop=True)
                                nc.tensor.matmul(
                                    E[:, CH[c]], selE_sb[:, m, :],
                                    mur[:, CH[c]], start=True, stop=True)
                            t1 = t1p.tile([128, N], f32, tag="t1")
                            nc.vector.tensor_mul(t1[:], raws[ml][:], A[:])
                            nc.vector.scalar_tensor_tensor(
                                out=qkn[m][:], in0=t1[:],
                                scalar=(lnbq[:] if side == 0 else lnbk[:]),
                                in1=E[:], op0=Alu.add, op1=Alu.add)

            # ---- V: natural layout + ones column -------------------------
            with ExitStack() as sctx:
                vps = sctx.enter_context(
                    tc.tile_pool(name="vps", bufs=4, space="PSUM"))
                wvsl = sctx.enter_context(tc.tile_pool(name="wvsl", bufs=2))
                for ci, (c0, cw) in enumerate(((0, 512), (512, 256))):
                    wvs = wvsl.tile([128, 6, cw], f32r, tag=f"wvs{ci}")
                    nc.sync.dma_start(out=wvs[:], in_=wv_r[:, :, c0:c0 + cw])
                    for t in range(8):
                        ps = vps.tile([128, cw], f32, tag="v")
                        for k in range(6):
                            nc.tensor.matmul(
                                ps[:], xt[k][:, t * 128:(t + 1) * 128],
                                wvs[:, k, :], start=(k == 0), stop=False)
                        nc.tensor.matmul(
                            ps[:], xt_ones[0:1, t * 128:(t + 1) * 128],
                            wv_brow[0:1, c0:c0 + cw], start=False, stop=True)
                        h0, nh = ci * 8, cw // 64
                        nc.vector.tensor_copy(
                            v_sb[t][:, h0:h0 + nh, 0:64],
                            ps[:].rearrange("p (h d) -> p h d", d=64))
                for t in range(8):
                    nc.sync.dma_start(
                        out=v_sb[t][:, :, 64:65],
                        in_=onesr[0:1, 0:12].to_broadcast((128, 12, 1)))

        # ---- ATT per head + PROJ (xt pool closed) ------------------------
        with ExitStack() as octx:
            outp = octx.enter_context(tc.tile_pool(name="outp", bufs=1))
            outh = [outp.tile([64, N], f32r, name=f"outh{h}", tag=f"outh{h}")
                    for h in range(12)]
            with ExitStack() as sctx:
                sps = sctx.enter_context(
                    tc.tile_pool(name="sps", bufs=2, space="PSUM"))
                avps = sctx.enter_context(
                    tc.tile_pool(name="avps", bufs=2, space="PSUM"))
                nbcps = sctx.enter_context(
                    tc.tile_pool(name="nbcps", bufs=2, space="PSUM"))
                attnp = sctx.enter_context(tc.tile_pool(name="attn", bufs=4))
                rdp = sctx.enter_context(tc.tile_pool(name="rd", bufs=2))
                avsbp = sctx.enter_context(tc.tile_pool(name="avsb", bufs=2))
                for h in range(12):
                    lo = (h & 1) * 64
                    hi = lo + 64
                    qn, kn = qkn[h // 2], qkn[6 + h // 2]
                    av = [avps.tile([65, 512], f32, name=f"av{h}_{i}", tag="av")
                          for i in range(2)]
                    for kt in range(8):
                        s = sps.tile([128, N], f32, tag="s")
                        for c in range(2):
                            nc.tensor.matmul(
                                s[:, CH[c]],
                                kn[lo:hi, kt * 128:(kt + 1) * 128],
                                qn[lo:hi, CH[c]], start=True, stop=True)
                        at = attnp.tile([128, N], f32r, tag="at")
                        nc.scalar.activation(out=at[:], in_=s[:], func=AF.Exp,
                                             scale=float(D) ** -0.5)
                        for c in range(2):
                            nc.tensor.matmul(av[c][:], v_sb[kt][:, h, :],
                                             at[:, CH[c]],
                                             start=(kt == 0), stop=(kt == 7))
                    rd = rdp.tile([65, N], f32r, tag="rd")
                    for c in range(2):
                        nc.vector.reciprocal(rd[64:65, CH[c]],
                                             av[c][64:65, :])
                    for c in range(2):
                        nbc = nbcps.tile([64, 512], f32, tag="nbc")
                        nc.tensor.matmul(nbc[:], ones64[64:65, :],
                                         rd[64:65, CH[c]],
                                         start=True, stop=True)
                        avs = avsbp.tile([64, 512], f32, tag="avs")
                        nc.vector.tensor_copy(avs[:], av[c][0:64, :])
                        nc.vector.tensor_mul(outh[h][:, CH[c]], avs[:],
                                             nbc[:])

            with ExitStack() as sctx:
                yps = sctx.enter_context(
                    tc.tile_pool(name="yps", bufs=4, space="PSUM"))
                wpsl = sctx.enter_context(tc.tile_pool(name="wpsl", bufs=2))
                ysbp = sctx.enter_context(tc.tile_pool(name="ysb", bufs=4))
                for m in range(6):
                    wps = wpsl.tile([64, 12, 128], f32r, tag="wps")
                    nc.sync.dma_start(
                        out=wps[:], in_=wp_h[:, :, m * 128:(m + 1) * 128])
                    for c in range(2):
                        ps = yps.tile([128, 512], f32, tag="y")
                        for k in range(12):
                            nc.tensor.matmul(ps[:], wps[:, k, :],
                                             outh[k][:, CH[c]],
                                             start=(k == 0), stop=False)
                        nc.tensor.matmul(
                            ps[:], wp_brow[0:1, m * 128:(m + 1) * 128],
                            xt_ones[0:1, CH[c]], start=False, stop=True)
                        ysb = ysbp.tile([128, 512], f32, tag="ysb")
                        nc.scalar.activation(out=ysb[:], in_=ps[:],
                                             func=AF.Copy)
                        nc.sync.dma_start(
                            out=yT[m * 128:(m + 1) * 128, CH[c]], in_=ysb[:])


def _host_inputs(x, qkv_w, qkv_b, qn_w, qn_b, kn_w, kn_b, proj_w, proj_b):
    f = np.float32
    ones_row = np.ones((1, N), f)
    wqk = np.concatenate([qkv_w[:2 * C].T, qkv_b[None, :2 * C]], 0).astype(f)
    wv = np.concatenate([qkv_w[2 * C:].T, qkv_b[None, 2 * C:]], 0).astype(f)
    wp = np.concatenate([proj_w.T, proj_b[None, :]], 0).astype(f)

    def sel(sign):
        s = np.zeros((12, 12, 128), f)
        for m in range(12):
            w_side = qn_w if m < 6 else kn_w
            ml = m % 6
            for p in range(128):
                s[m, 2 * ml + p // 64, p] = sign * w_side[p % 64]
        return s

    sumsel = np.zeros((6, 128, 12), f)
    for ml in range(6):
        for p in range(128):
            sumsel[ml, p, 2 * ml + p // 64] = 1.0
    lnb = np.stack([np.tile(qn_b, 2)[:, None],
                    np.tile(kn_b, 2)[:, None]]).astype(f)
    shared = dict(wqk=wqk, wv=wv, wp=wp, selA=sel(1.0), selE=sel(-1.0),
                  sumsel=sumsel, lnb=lnb, onesr=np.ones((1, 64), f))
    in_maps = []
    for b in range(B):
        m = dict(shared)
        m["xT"] = np.concatenate([x[b].T, ones_row], 0).astype(f)
        in_maps.append(m)
    return in_maps


def run(inputs, loop_k=1):
    from concourse.bass_utils import run_bass_kernel_spmd
    nc = _build(loop_k)
    in_maps = _host_inputs(**{k: np.asarray(v, np.float32)
                              for k, v in inputs.items()})
    res = run_bass_kernel_spmd(nc, in_maps, list(range(NCORES)))
    out = np.empty((B, N, C), np.float32)
    for b in range(B):
        out[b] = res.results[b]["yT"].T
    return out


def kernel(**inputs):
    return run(inputs, loop_k=1)


# revision 7
# speedup vs baseline: 10.8712x; 10.8712x over previous
"""Trainium2 Bass kernel for nn_Attention_80212809220238.

Data-parallel over batch (B=8 -> 1 batch element per NeuronCore).
Per core, full attention layer on [1024, 768] in transposed layouts:
  - qkT = Wqk_aug @ xT_aug   (q,k in [channels, tokens]; biases via ones-row)
  - LN over head_dim in transposed layout: per-token stats via PE selector
    matmuls; rank-1 broadcasts A = w(x)r and E = -w(x)mu*r materialized by
    K=12 selector matmuls; applied in 2 DVE passes.
  - v in natural layout [tokens, channels] + ones column -> v_aug
  - scoresT = k~^T q~ per head; exp on ACT with 1/sqrt(D) folded into scale
    (no max-subtraction: LN bounds |scores|/8 <= 8, exp safe in fp32)
  - out.T = v_aug^T @ attnT: row 64 = softmax denominator for free
  - normalize via K=1 ones broadcast matmul + DVE multiply
  - y.T = Wp_aug @ outT (bias via ones row), DMA straight to DRAM
All matmuls float32r (fp22 multiply, fp32 accumulate, full PE rate).
"""

import sys
from contextlib import ExitStack

import numpy as np

if "/opt/trn_rl_repo" not in sys.path:
    sys.path.insert(0, "/opt/trn_rl_repo")

B, N, C = 8, 1024, 768
H, D = 12, 64
EPS = 1e-5
NCORES = 8
CH = (slice(0, 512), slice(512, 1024))  # token chunks

_CACHE: dict = {}


def _build(loop_k: int = 1):
    key = ("nc", loop_k)
    if key in _CACHE:
        return _CACHE[key]
    import concourse.mybir as mybir
    import concourse.tile as tile
    from concourse import bacc

    f32, f32r = mybir.dt.float32, mybir.dt.float32r

    nc = bacc.Bacc("TRN2", target_bir_lowering=False, debug=False,
                   num_devices=NCORES)
    prm = dict(
        xT=nc.declare_dram_parameter("xT", [C + 1, N], f32r, isOutput=False),
        wqk=nc.declare_dram_parameter("wqk", [C + 1, 2 * C], f32r,
                                      isOutput=False),
        wv=nc.declare_dram_parameter("wv", [C + 1, H * 65], f32r,
                                     isOutput=False),
        wp=nc.declare_dram_parameter("wp", [C + 1, C], f32r, isOutput=False),
        selA=nc.declare_dram_parameter("selA", [12, 12, 128], f32r,
                                       isOutput=False),
        selE=nc.declare_dram_parameter("selE", [12, 12, 128], f32r,
                                       isOutput=False),
        sumsel=nc.declare_dram_parameter("sumsel", [128, 72], f32r,
                                         isOutput=False),
        lnb=nc.declare_dram_parameter("lnb", [2, 128, 1], f32, isOutput=False),
        onesr=nc.declare_dram_parameter("onesr", [1, 64], f32r,
                                        isOutput=False),
        yT=nc.declare_dram_parameter("yT", [C, N], f32, isOutput=True),
    )

    with tile.TileContext(nc) as tc:
        with nc.allow_low_precision("fp32r attention"):
            if loop_k == 1:
                _emit(nc, tc, prm)
            else:
                with tc.For_i(0, loop_k, 1) as _:
                    _emit(nc, tc, prm)
    nc.compile()
    _CACHE[key] = nc
    return nc


def _emit(nc, tc, prm):
    import concourse.mybir as mybir

    f32, f32r = mybir.dt.float32, mybir.dt.float32r
    AF = mybir.ActivationFunctionType
    Alu = mybir.AluOpType
    xT, wqk, wv, wp = prm["xT"], prm["wqk"], prm["wv"], prm["wp"]
    selA, selE, sumsel, lnb, yT = (prm["selA"], prm["selE"], prm["sumsel"],
                                   prm["lnb"], prm["yT"])
    onesr = prm["onesr"]

    wqk_r = wqk[0:C, :].rearrange("(t p) c -> p t c", p=128)
    wp_h = wp[0:C, :].rearrange("(h d) c -> d h c", d=64)

    with ExitStack() as ctx:
        consts = ctx.enter_context(tc.tile_pool(name="consts", bufs=1))
        selA_sb = consts.tile([12, 12, 128], f32r, tag="selA")
        nc.sync.dma_start(out=selA_sb[:], in_=selA[:])
        selE_sb = consts.tile([12, 12, 128], f32r, tag="selE")
        nc.sync.dma_start(out=selE_sb[:], in_=selE[:])
        sumsel_sb = consts.tile([128, 6, 12], f32r, tag="sumsel")
        nc.sync.dma_start(out=sumsel_sb[:],
                          in_=sumsel[:].rearrange("p (l j) -> p l j", j=12))
        lnbq = consts.tile([128, 1], f32, tag="lnbq")
        nc.sync.dma_start(out=lnbq[:], in_=lnb[0])
        lnbk = consts.tile([128, 1], f32, tag="lnbk")
        nc.sync.dma_start(out=lnbk[:], in_=lnb[1])
        wqk_brow = consts.tile([1, 2 * C], f32r, tag="wqk_brow")
        nc.sync.dma_start(out=wqk_brow[:], in_=wqk[C:C + 1, :])
        wv_brow = consts.tile([1, H * 65], f32r, tag="wv_brow")
        nc.sync.dma_start(out=wv_brow[:], in_=wv[C:C + 1, :])
        wp_brow = consts.tile([1, C], f32r, tag="wp_brow")
        nc.sync.dma_start(out=wp_brow[:], in_=wp[C:C + 1, :])
        ones64 = consts.tile([65, 64], f32r, tag="ones64")
        nc.sync.dma_start(out=ones64[64:65, :], in_=onesr[0:1, :])
        eps12 = consts.tile([12, 1], f32, tag="eps12")
        nc.vector.memset(eps12[:], EPS)
        xt_ones = consts.tile([1, N], f32r, tag="xt_ones")
        nc.sync.dma_start(out=xt_ones[:], in_=xT[C:C + 1, :])

        qknp = ctx.enter_context(tc.tile_pool(name="qknp", bufs=1))
        qkn = [qknp.tile([128, N], f32r, name=f"qkn{m}", tag=f"qkn{m}")
               for m in range(12)]
        vp = ctx.enter_context(tc.tile_pool(name="vp", bufs=1))
        v_sb = [vp.tile([128, H * 65], f32r, name=f"v{t}", tag=f"v{t}")
                for t in range(8)]

        with ExitStack() as xctx:
            xp = xctx.enter_context(tc.tile_pool(name="xp", bufs=1))
            xt = [xp.tile([128, N], f32r, name=f"xt{k}", tag=f"xt{k}")
                  for k in range(6)]
            for k in range(6):
                nc.sync.dma_start(out=xt[k][:],
                                  in_=xT[k * 128:(k + 1) * 128, :])

            # ---- QK + LN, one side at a time (q: m=0..5, k: m=6..11) -----
            for side in range(2):
                m0 = side * 6
                with ExitStack() as sctx:
                    rawp = sctx.enter_context(
                        tc.tile_pool(name=f"raw{side}", bufs=6))
                    stsb = sctx.enter_context(
                        tc.tile_pool(name=f"stsb{side}", bufs=1))
                    s1p = stsb.tile([12, N], f32, tag="s1p")
                    s2p = stsb.tile([12, N], f32, tag="s2p")
                    raws = []
                    with ExitStack() as mctx:
                        qkps = mctx.enter_context(tc.tile_pool(
                            name=f"qkps{side}", bufs=2, space="PSUM"))
                        stps = mctx.enter_context(tc.tile_pool(
                            name=f"stps{side}", bufs=1, space="PSUM"))
                        wsl = mctx.enter_context(
                            tc.tile_pool(name=f"wsl{side}", bufs=1))
                        wsb = [wsl.tile([128, 6 * 128], f32r,
                                        name=f"wsb{side}_{k}", tag=f"w{k}")
                               for k in range(6)]
                        for k in range(6):
                            nc.sync.dma_start(
                                out=wsb[k][:],
                                in_=wqk[k * 128:(k + 1) * 128,
                                        m0 * 128:(m0 + 6) * 128])
                        sqp = mctx.enter_context(
                            tc.tile_pool(name=f"sq{side}", bufs=2))
                        stt = [stps.tile([12, 512], f32, name=f"st{side}_{j}",
                                         tag=f"st{j}") for j in range(4)]
                        for ml in range(6):
                            m = m0 + ml
                            ps = qkps.tile([128, N], f32, tag="qk")
                            for c in range(2):
                                for k in range(6):
                                    nc.tensor.matmul(
                                        ps[:, CH[c]],
                                        wsb[k][:, ml * 128:(ml + 1) * 128],
                                        xt[k][:, CH[c]],
                                        start=(k == 0), stop=False)
                                nc.tensor.matmul(
                                    ps[:, CH[c]],
                                    wqk_brow[0:1, m * 128:(m + 1) * 128],
                                    xt_ones[0:1, CH[c]], start=False,
                                    stop=True)
                            raw = rawp.tile([128, N], f32r, tag="raw")
                            raws.append(raw)
                            nc.scalar.activation(out=raw[:], in_=ps[:],
                                                 func=AF.Copy)
                            sq = sqp.tile([128, N], f32r, tag="sq")
                            nc.vector.tensor_mul(sq[:], raw[:], raw[:])
                            # accumulate per-token sums into packed [12, 512]
                            # rows via one-hot selector columns
                            for c in range(2):
                                nc.tensor.matmul(
                                    stt[c][:], sumsel_sb[:, ml, :],
                                    raw[:, CH[c]],
                                    start=(ml == 0), stop=(ml == 5))
                                nc.tensor.matmul(
                                    stt[2 + c][:], sumsel_sb[:, ml, :],
                                    sq[:, CH[c]],
                                    start=(ml == 0), stop=(ml == 5))
                        for c in range(2):
                            nc.vector.tensor_copy(s1p[:, CH[c]], stt[c][:])
                            nc.vector.tensor_copy(s2p[:, CH[c]], stt[2 + c][:])

                    # packed LN stats on [12, N] tiles
                    tsq = stsb.tile([12, N], f32, tag="tsq")
                    nc.vector.tensor_mul(tsq[:], s1p[:], s1p[:])
                    ve = stsb.tile([12, N], f32, tag="ve")
                    nc.vector.scalar_tensor_tensor(
                        out=ve[:], in0=tsq[:], scalar=1.0 / 64.0, in1=s2p[:],
                        op0=Alu.mult, op1=Alu.subtract)  # S1^2/64 - S2
                    std = stsb.tile([12, N], f32, tag="std")
                    nc.scalar.activation(out=std[:], in_=ve[:], func=AF.Sqrt,
                                         bias=eps12[:], scale=-1.0 / 64.0)
                    r_pk = stsb.tile([12, N], f32r, tag="r_pk")
                    nc.vector.reciprocal(r_pk[:], std[:])
                    mur = stsb.tile([12, N], f32r, tag="mur")
                    nc.vector.scalar_tensor_tensor(
                        out=mur[:], in0=s1p[:], scalar=1.0 / 64.0, in1=r_pk[:],
                        op0=Alu.mult, op1=Alu.mult)

                    # apply: qkn = raw*(w(x)r) + lnb + (-w(x)mu r)
                    with ExitStack() as actx:
                        aps = actx.enter_context(tc.tile_pool(
                            name=f"aps{side}", bufs=2, space="PSUM"))
                        eps_ = actx.enter_context(tc.tile_pool(
                            name=f"eps{side}", bufs=2, space="PSUM"))
                        t1p = actx.enter_context(
                            tc.tile_pool(name=f"t1{side}", bufs=2))
                        for ml in range(6):
                            m = m0 + ml
                            A = aps.tile([128, N], f32, tag="A")
                            E = eps_.tile([128, N], f32, tag="E")
                            for c in range(2):
                                nc.tensor.matmul(
                                    A[:, CH[c]], selA_sb[:, m, :],
                                    r_pk[:, CH[c]], start=True, stop=True)
                                nc.tensor.matmul(
                                    E[:, CH[c]], selE_sb[:, m, :],
                                    mur[:, CH[c]], start=True, stop=True)
                            t1 = t1p.tile([128, N], f32, tag="t1")
                            nc.vector.tensor_mul(t1[:], raws[ml][:], A[:])
                            nc.vector.scalar_tensor_tensor(
                                out=qkn[m][:], in0=t1[:],
                                scalar=(lnbq[:] if side == 0 else lnbk[:]),
                                in1=E[:], op0=Alu.add, op1=Alu.add)

            # ---- V: natural layout, ones columns via augmented weights --
            with ExitStack() as sctx:
                vps = sctx.enter_context(
                    tc.tile_pool(name="vps", bufs=4, space="PSUM"))
                wvsl = sctx.enter_context(tc.tile_pool(name="wvsl", bufs=1))
                wvsb = [wvsl.tile([128, H * 65], f32r, name=f"wvsb{k}",
                                  tag=f"wv{k}") for k in range(6)]
                for k in range(6):
                    nc.sync.dma_start(out=wvsb[k][:],
                                      in_=wv[k * 128:(k + 1) * 128, :])
                for c0, cw in ((0, 512), (512, 268)):
                    for t in range(8):
                        ps = vps.tile([128, cw], f32, tag="v")
                        for k in range(6):
                            nc.tensor.matmul(
                                ps[:], xt[k][:, t * 128:(t + 1) * 128],
                                wvsb[k][:, c0:c0 + cw],
                                start=(k == 0), stop=False)
                        nc.tensor.matmul(
                            ps[:], xt_ones[0:1, t * 128:(t + 1) * 128],
                            wv_brow[0:1, c0:c0 + cw], start=False, stop=True)
                        nc.vector.tensor_copy(v_sb[t][:, c0:c0 + cw], ps[:])

        # ---- ATT per head + PROJ (xt pool closed) ------------------------
        with ExitStack() as octx:
            outp = octx.enter_context(tc.tile_pool(name="outp", bufs=1))
            outh = [outp.tile([64, N], f32r, name=f"outh{h}", tag=f"outh{h}")
                    for h in range(12)]
            with ExitStack() as sctx:
                sps = sctx.enter_context(
                    tc.tile_pool(name="sps", bufs=2, space="PSUM"))
                avps = sctx.enter_context(
                    tc.tile_pool(name="avps", bufs=2, space="PSUM"))
                nbcps = sctx.enter_context(
                    tc.tile_pool(name="nbcps", bufs=2, space="PSUM"))
                attnp = sctx.enter_context(tc.tile_pool(name="attn", bufs=4))
                rdp = sctx.enter_context(tc.tile_pool(name="rd", bufs=2))
                avsbp = sctx.enter_context(tc.tile_pool(name="avsb", bufs=2))
                for h in range(12):
                    lo = (h & 1) * 64
                    hi = lo + 64
                    qn, kn = qkn[h // 2], qkn[6 + h // 2]
                    av = [avps.tile([65, 512], f32, name=f"av{h}_{i}", tag="av")
                          for i in range(2)]
                    for kt in range(8):
                        s = sps.tile([128, N], f32, tag="s")
                        for c in range(2):
                            nc.tensor.matmul(
                                s[:, CH[c]],
                                kn[lo:hi, kt * 128:(kt + 1) * 128],
                                qn[lo:hi, CH[c]], start=True, stop=True)
                        at = attnp.tile([128, N], f32r, tag="at")
                        nc.scalar.activation(out=at[:], in_=s[:], func=AF.Exp,
                                             scale=float(D) ** -0.5)
                        for c in range(2):
                            nc.tensor.matmul(av[c][:],
                                             v_sb[kt][:, h * 65:h * 65 + 65],
                                             at[:, CH[c]],
                                             start=(kt == 0), stop=(kt == 7))
                    rd = rdp.tile([65, N], f32r, tag="rd")
                    for c in range(2):
                        nc.vector.reciprocal(rd[64:65, CH[c]],
                                             av[c][64:65, :])
                    for c in range(2):
                        nbc = nbcps.tile([64, 512], f32, tag="nbc")
                        nc.tensor.matmul(nbc[:], ones64[64:65, :],
                                         rd[64:65, CH[c]],
                                         start=True, stop=True)
                        avs = avsbp.tile([64, 512], f32, tag="avs")
                        nc.vector.tensor_copy(avs[:], av[c][0:64, :])
                        nc.vector.tensor_mul(outh[h][:, CH[c]], avs[:],
                                             nbc[:])

            with ExitStack() as sctx:
                yps = sctx.enter_context(
                    tc.tile_pool(name="yps", bufs=4, space="PSUM"))
                wpsl = sctx.enter_context(tc.tile_pool(name="wpsl", bufs=1))
                ysbp = sctx.enter_context(tc.tile_pool(name="ysb", bufs=4))
                wps_all = wpsl.tile([64, 12, C], f32r, tag="wps")
                nc.sync.dma_start(out=wps_all[:], in_=wp_h[:])
                for m in range(6):
                    wps = wps_all[:, :, m * 128:(m + 1) * 128]
                    for c in range(2):
                        ps = yps.tile([128, 512], f32, tag="y")
                        for k in range(12):
                            nc.tensor.matmul(ps[:], wps[:, k, :],
                                             outh[k][:, CH[c]],
                                             start=(k == 0), stop=False)
                        nc.tensor.matmul(
                            ps[:], wp_brow[0:1, m * 128:(m + 1) * 128],
                            xt_ones[0:1, CH[c]], start=False, stop=True)
                        ysb = ysbp.tile([128, 512], f32, tag="ysb")
                        nc.scalar.activation(out=ysb[:], in_=ps[:],
                                             func=AF.Copy)
                        nc.sync.dma_start(
                            out=yT[m * 128:(m + 1) * 128, CH[c]], in_=ysb[:])


def _host_inputs(x, qkv_w, qkv_b, qn_w, qn_b, kn_w, kn_b, proj_w, proj_b):
    f = np.float32
    ones_row = np.ones((1, N), f)
    wqk = np.concatenate([qkv_w[:2 * C].T, qkv_b[None, :2 * C]], 0).astype(f)
    wv = np.zeros((C + 1, H * 65), f)
    for h in range(H):
        wv[0:C, h * 65:h * 65 + 64] = qkv_w[2 * C + h * 64:
                                            2 * C + (h + 1) * 64].T
        wv[C, h * 65:h * 65 + 64] = qkv_b[2 * C + h * 64:2 * C + (h + 1) * 64]
        wv[C, h * 65 + 64] = 1.0
    wp = np.concatenate([proj_w.T, proj_b[None, :]], 0).astype(f)

    def sel(sign):
        s = np.zeros((12, 12, 128), f)  # [k, m, p]
        for m in range(12):
            w_side = qn_w if m < 6 else kn_w
            ml = m % 6
            for p in range(128):
                s[2 * ml + p // 64, m, p] = sign * w_side[p % 64]
        return s

    sumsel = np.zeros((128, 6, 12), f)
    for ml in range(6):
        for p in range(128):
            sumsel[p, ml, 2 * ml + p // 64] = 1.0
    sumsel = sumsel.reshape(128, 72)
    lnb = np.stack([np.tile(qn_b, 2)[:, None],
                    np.tile(kn_b, 2)[:, None]]).astype(f)
    shared = dict(wqk=wqk, wv=wv, wp=wp, selA=sel(1.0), selE=sel(-1.0),
                  sumsel=sumsel, lnb=lnb, onesr=np.ones((1, 64), f))
    in_maps = []
    for b in range(B):
        m = dict(shared)
        m["xT"] = np.concatenate([x[b].T, ones_row], 0).astype(f)
        in_maps.append(m)
    return in_maps


def run(inputs, loop_k=1):
    from concourse.bass_utils import run_bass_kernel_spmd
    nc = _build(loop_k)
    in_maps = _host_inputs(**{k: np.asarray(v, np.float32)
                              for k, v in inputs.items()})
    res = run_bass_kernel_spmd(nc, in_maps, list(range(NCORES)))
    out = np.empty((B, N, C), np.float32)
    for b in range(B):
        out[b] = res.results[b]["yT"].T
    return out


def kernel(**inputs):
    return run(inputs, loop_k=1)


# revision 13
# speedup vs baseline: 11.4944x; 1.0573x over previous
"""Trainium2 Bass kernel for nn_Attention_80212809220238.

Data-parallel over batch (B=8 -> 1 batch element per NeuronCore).
Per core, full attention layer on [1024, 768] in transposed layouts:
  - qkT = Wqk_aug @ xT_aug   (q,k in [channels, tokens]; biases via ones-row)
  - LN over head_dim in transposed layout: per-token stats via PE selector
    matmuls; rank-1 broadcasts A = w(x)r and E = -w(x)mu*r materialized by
    K=12 selector matmuls; applied in 2 DVE passes.
  - v in natural layout [tokens, channels] + ones column -> v_aug
  - scoresT = k~^T q~ per head; exp on ACT with 1/sqrt(D) folded into scale
    (no max-subtraction: LN bounds |scores|/8 <= 8, exp safe in fp32)
  - out.T = v_aug^T @ attnT: row 64 = softmax denominator for free
  - normalize via K=1 ones broadcast matmul + DVE multiply
  - y.T = Wp_aug @ outT (bias via ones row), DMA straight to DRAM
All matmuls float32r (fp22 multiply, fp32 accumulate, full PE rate).
"""

import sys
from contextlib import ExitStack

import numpy as np

if "/opt/trn_rl_repo" not in sys.path:
    sys.path.insert(0, "/opt/trn_rl_repo")

B, N, C = 8, 1024, 768
H, D = 12, 64
EPS = 1e-5
NCORES = 8
CH = (slice(0, 512), slice(512, 1024))  # token chunks

_CACHE: dict = {}


def _build(loop_k: int = 1, trace_sim: bool = False):
    key = ("nc", loop_k, trace_sim)
    if key in _CACHE:
        return _CACHE[key]
    import concourse.mybir as mybir
    import concourse.tile as tile
    from concourse import bacc

    f32, f32r = mybir.dt.float32, mybir.dt.float32r

    nc = bacc.Bacc("TRN2", target_bir_lowering=False, debug=False,
                   num_devices=NCORES)
    prm = dict(
        xT=nc.declare_dram_parameter("xT", [C + 1, N], f32r, isOutput=False),
        wqk=nc.declare_dram_parameter("wqk", [C + 1, 2 * C], f32r,
                                      isOutput=False),
        wv=nc.declare_dram_parameter("wv", [C + 1, H * 65], f32r,
                                     isOutput=False),
        wp=nc.declare_dram_parameter("wp", [C + 1, C], f32r, isOutput=False),
        selA=nc.declare_dram_parameter("selA", [12, 12, 128], f32r,
                                       isOutput=False),
        selE=nc.declare_dram_parameter("selE", [12, 12, 128], f32r,
                                       isOutput=False),
        sumsel=nc.declare_dram_parameter("sumsel", [128, 72], f32r,
                                         isOutput=False),
        lnb=nc.declare_dram_parameter("lnb", [2, 128, 1], f32, isOutput=False),
        onesr=nc.declare_dram_parameter("onesr", [1, 64], f32r,
                                        isOutput=False),
        yT=nc.declare_dram_parameter("yT", [C, N], f32, isOutput=True),
    )

    with tile.TileContext(nc, trace_sim=trace_sim) as tc:
        with nc.allow_low_precision("fp32r attention"):
            if loop_k == 1:
                _emit(nc, tc, prm)
            else:
                with tc.For_i(0, loop_k, 1) as _:
                    _emit(nc, tc, prm)
    nc.compile()
    _CACHE[key] = nc
    return nc


def _emit(nc, tc, prm):
    import concourse.mybir as mybir

    f32, f32r = mybir.dt.float32, mybir.dt.float32r
    AF = mybir.ActivationFunctionType
    Alu = mybir.AluOpType
    xT, wqk, wv, wp = prm["xT"], prm["wqk"], prm["wv"], prm["wp"]
    selA, selE, sumsel, lnb, yT = (prm["selA"], prm["selE"], prm["sumsel"],
                                   prm["lnb"], prm["yT"])
    onesr = prm["onesr"]

    wp_h = wp[0:C, :].rearrange("(h d) c -> d h c", d=64)

    with ExitStack() as ctx:
        consts = ctx.enter_context(tc.tile_pool(name="consts", bufs=1))
        qkc_es = ExitStack()

        def load_consts():
            d = {}
            d["selA_sb"] = qkc.tile([12, 12, 128], f32r, name="selA_sb",
                                    tag="selA")
            nc.sync.dma_start(out=d["selA_sb"][:], in_=selA[:])
            d["selE_sb"] = qkc.tile([12, 12, 128], f32r, name="selE_sb",
                                    tag="selE")
            nc.sync.dma_start(out=d["selE_sb"][:], in_=selE[:])
            d["sumsel_sb"] = qkc.tile([128, 6, 12], f32r, name="sumsel_sb",
                                      tag="sumsel")
            nc.sync.dma_start(
                out=d["sumsel_sb"][:],
                in_=sumsel[:].rearrange("p (l j) -> p l j", j=12))
            d["lnbq"] = qkc.tile([128, 1], f32, name="lnbq", tag="lnbq")
            nc.sync.dma_start(out=d["lnbq"][:], in_=lnb[0])
            d["lnbk"] = qkc.tile([128, 1], f32, name="lnbk", tag="lnbk")
            nc.sync.dma_start(out=d["lnbk"][:], in_=lnb[1])
            d["wv_brow"] = consts.tile([1, H * 65], f32r, name="wv_brow",
                                       tag="wv_brow")
            nc.sync.dma_start(out=d["wv_brow"][:], in_=wv[C:C + 1, :])
            d["wp_brow"] = consts.tile([1, C], f32r, name="wp_brow",
                                       tag="wp_brow")
            nc.sync.dma_start(out=d["wp_brow"][:], in_=wp[C:C + 1, :])
            d["ones64"] = consts.tile([65, 64], f32r, name="ones64",
                                      tag="ones64")
            nc.sync.dma_start(out=d["ones64"][64:65, :], in_=onesr[0:1, :])
            d["eps12"] = qkc.tile([12, 1], f32, name="eps12", tag="eps12")
            nc.vector.memset(d["eps12"][:], EPS)
            return d

        qknp = ctx.enter_context(tc.tile_pool(name="qknp", bufs=1))
        qkn = [qknp.tile([128, N], f32r, name=f"qkn{m}", tag=f"qkn{m}")
               for m in range(12)]
        vp = ctx.enter_context(tc.tile_pool(name="vp", bufs=1))
        v_sb = [vp.tile([128, H * 65], f32r, name=f"v{t}", tag=f"v{t}")
                for t in range(8)]

        xp_es = ExitStack()
        xp = xp_es.enter_context(tc.tile_pool(name="xp", bufs=1))
        xt = [xp.tile([128, N], f32r, name=f"xt{k}", tag=f"xt{k}")
              for k in range(6)]

        # qkc opens after xp so it can close before xp (LIFO)
        qkc = qkc_es.enter_context(tc.tile_pool(name="qkc", bufs=1))
        wsl_es = ExitStack()
        wslp = wsl_es.enter_context(tc.tile_pool(name="wsl", bufs=1))
        wv_es = ExitStack()
        wvpool = wv_es.enter_context(tc.tile_pool(name="wvp", bufs=1))
        wsb = [None, None]

        def load_wsb(side):
            wsb[side] = [wslp.tile([128, 6 * 128], f32r,
                                   name=f"wsb{side}_{k}", tag=f"w{k}")
                         for k in range(6)]
            for k in range(6):
                nc.sync.dma_start(
                    out=wsb[side][k][:],
                    in_=wqk[k * 128:(k + 1) * 128,
                            side * 768:(side + 1) * 768])

        xt_ones = consts.tile([1, N], f32r, name="xt_ones", tag="xt_ones")
        wqk_brow = qkc.tile([1, 2 * C], f32r, name="wqk_brow",
                            tag="wqk_brow")
        load_wsb(0)
        for k in range(6):
            nc.sync.dma_start(out=xt[k][:, CH[0]],
                              in_=xT[k * 128:(k + 1) * 128, CH[0]])
            nc.sync.dma_start(out=xt[k][:, CH[1]],
                              in_=xT[k * 128:(k + 1) * 128, CH[1]])
            if k == 0:
                nc.sync.dma_start(out=xt_ones[:], in_=xT[C:C + 1, :])
                nc.sync.dma_start(out=wqk_brow[:], in_=wqk[C:C + 1, :])
        d = load_consts()

        # v-weight tiles: chunk A (cols 0:512) now, chunk B later
        VCH = ((0, 512), (512, 268))
        wvsb = [None, None]

        def load_wv(ci):
            c0, cw = VCH[ci]
            wvsb[ci] = [wvpool.tile([128, 512], f32r, name=f"wvsb{ci}_{k}",
                                    tag=f"wv{k}") for k in range(6)]
            for k in range(6):
                nc.sync.dma_start(out=wvsb[ci][k][:, 0:cw],
                                  in_=wv[k * 128:(k + 1) * 128, c0:c0 + cw])

        def emit_v(ci, t, vps):
            c0, cw = VCH[ci]
            ps = vps.tile([128, cw], f32, tag="v")
            for k in range(6):
                nc.tensor.matmul(ps[:], xt[k][:, t * 128:(t + 1) * 128],
                                 wvsb[ci][k][:, 0:cw],
                                 start=(k == 0), stop=False)
            nc.tensor.matmul(ps[:], xt_ones[0:1, t * 128:(t + 1) * 128],
                             d["wv_brow"][0:1, c0:c0 + cw],
                             start=False, stop=True)
            nc.vector.tensor_copy(v_sb[t][:, c0:c0 + cw], ps[:])

        # ================= QK sides ======================================
        for side in range(2):
            m0 = side * 6
            with ExitStack() as sctx:
                stsb = sctx.enter_context(
                    tc.tile_pool(name=f"stsb{side}", bufs=1))
                s1p = stsb.tile([12, N], f32, name=f"s1p{side}", tag="s1p")
                s2p = stsb.tile([12, N], f32, name=f"s2p{side}", tag="s2p")
                with ExitStack() as mctx:
                    qkps = mctx.enter_context(tc.tile_pool(
                        name=f"qkps{side}", bufs=2, space="PSUM"))
                    stps = mctx.enter_context(tc.tile_pool(
                        name=f"stps{side}", bufs=1, space="PSUM"))
                    sqp = mctx.enter_context(
                        tc.tile_pool(name=f"sq{side}", bufs=2))
                    stt = [stps.tile([12, 512], f32, name=f"st{side}_{j}",
                                     tag=f"st{j}") for j in range(4)]
                    for ml in range(6):
                        m = m0 + ml
                        ps = qkps.tile([128, N], f32, tag="qk")
                        for c in range(2):
                            for k in range(6):
                                nc.tensor.matmul(
                                    ps[:, CH[c]],
                                    wsb[side][k][:, ml * 128:(ml + 1) * 128],
                                    xt[k][:, CH[c]],
                                    start=(k == 0), stop=False)
                            nc.tensor.matmul(
                                ps[:, CH[c]],
                                wqk_brow[0:1, m * 128:(m + 1) * 128],
                                xt_ones[0:1, CH[c]], start=False, stop=True)
                        raw = qkn[m]
                        nc.scalar.activation(out=raw[:], in_=ps[:],
                                             func=AF.Copy)
                        sq = sqp.tile([128, N], f32r, tag="sq")
                        nc.vector.tensor_mul(sq[:], raw[:], raw[:])
                        for c in range(2):
                            nc.tensor.matmul(
                                stt[c][:], d["sumsel_sb"][:, ml, :],
                                raw[:, CH[c]],
                                start=(ml == 0), stop=(ml == 5))
                            nc.tensor.matmul(
                                stt[2 + c][:], d["sumsel_sb"][:, ml, :],
                                sq[:, CH[c]],
                                start=(ml == 0), stop=(ml == 5))
                        if side == 0 and ml == 4:
                            load_wv(0)    # v chunk-A weights for V filler
                        if side == 1 and ml == 3:
                            load_wv(1)
                    for c in range(2):
                        nc.vector.tensor_copy(s1p[:, CH[c]], stt[c][:])
                        nc.vector.tensor_copy(s2p[:, CH[c]], stt[2 + c][:])
                if side == 0:
                    load_wsb(1)   # k-side weights, reusing freed slots

                # V front (PE filler while packed stats math runs on DVE)
                with ExitStack() as actx:
                    vps = actx.enter_context(tc.tile_pool(
                        name=f"vps{side}", bufs=2, space="PSUM"))
                    ae = actx.enter_context(tc.tile_pool(
                        name=f"ae{side}", bufs=3, space="PSUM"))
                    t1p = actx.enter_context(
                        tc.tile_pool(name=f"t1{side}", bufs=2))
                    emit_v(side, 0, vps)
                    emit_v(side, 1, vps)

                    # packed LN stats on [12, N]
                    tsq = stsb.tile([12, N], f32, name=f"tsq{side}",
                                    tag="tmp", bufs=2)
                    nc.vector.tensor_mul(tsq[:], s1p[:], s1p[:])
                    ve = s2p
                    nc.vector.scalar_tensor_tensor(
                        out=ve[:], in0=tsq[:], scalar=1.0 / 64.0, in1=s2p[:],
                        op0=Alu.mult, op1=Alu.subtract)  # S1^2/64 - S2
                    std = stsb.tile([12, N], f32, name=f"std{side}",
                                    tag="tmp", bufs=2)
                    nc.scalar.activation(out=std[:], in_=ve[:], func=AF.Sqrt,
                                         bias=d["eps12"][:], scale=-1.0 / 64.0)
                    r_pk = stsb.tile([12, N], f32r, name=f"r_pk{side}",
                                     tag="r_pk")
                    nc.vector.reciprocal(r_pk[:], std[:])
                    mur = stsb.tile([12, N], f32r, name=f"mur{side}",
                                    tag="mur")
                    nc.vector.scalar_tensor_tensor(
                        out=mur[:], in0=s1p[:], scalar=1.0 / 64.0, in1=r_pk[:],
                        op0=Alu.mult, op1=Alu.mult)

                    # interleave remaining V tiles with the LN apply
                    for i in range(6):
                        emit_v(side, 2 + i, vps)
                        m = m0 + i
                        A = ae.tile([128, N], f32, tag="ae")
                        E = ae.tile([128, N], f32, tag="ae")
                        for c in range(2):
                            nc.tensor.matmul(A[:, CH[c]],
                                             d["selA_sb"][:, m, :],
                                             r_pk[:, CH[c]],
                                             start=True, stop=True)
                            nc.tensor.matmul(E[:, CH[c]],
                                             d["selE_sb"][:, m, :],
                                             mur[:, CH[c]],
                                             start=True, stop=True)
                        t1 = t1p.tile([128, N], f32, tag="t1")
                        nc.vector.tensor_mul(t1[:], qkn[m][:], A[:])
                        nc.vector.scalar_tensor_tensor(
                            out=qkn[m][:], in0=t1[:],
                            scalar=(d["lnbq"][:] if side == 0
                                    else d["lnbk"][:]),
                            in1=E[:], op0=Alu.add, op1=Alu.add)
        wv_es.close()
        wsl_es.close()
        qkc_es.close()
        xp_es.close()

        # ================= ATT per head + PROJ ===========================
        with ExitStack() as octx:
            outp = octx.enter_context(tc.tile_pool(name="outp", bufs=1))
            outh = [outp.tile([64, N], f32r, name=f"outh{h}", tag=f"outh{h}")
                    for h in range(12)]
            wpsl = octx.enter_context(tc.tile_pool(name="wpsl", bufs=1))
            wps_all = wpsl.tile([64, 12, C], f32r, name="wps_all", tag="wps")
            nc.sync.dma_start(out=wps_all[:, :, 0:384], in_=wp_h[:, :, 0:384])
            with ExitStack() as sctx:
                sps = sctx.enter_context(
                    tc.tile_pool(name="sps", bufs=2, space="PSUM"))
                avps = sctx.enter_context(
                    tc.tile_pool(name="avps", bufs=2, space="PSUM"))
                nbcps = sctx.enter_context(
                    tc.tile_pool(name="nbcps", bufs=2, space="PSUM"))
                attnp = sctx.enter_context(tc.tile_pool(name="attn", bufs=4))
                rdp = sctx.enter_context(tc.tile_pool(name="rd", bufs=2))
                for h in range(12):
                    lo = (h & 1) * 64
                    hi = lo + 64
                    qn, kn = qkn[h // 2], qkn[6 + h // 2]
                    av = [avps.tile([65, 512], f32, name=f"av{h}_{i}",
                                    tag="av") for i in range(2)]
                    for kt in range(8):
                        sl = sps.tile([128, N], f32, tag="s")
                        for c in range(2):
                            nc.tensor.matmul(
                                sl[:, CH[c]],
                                kn[lo:hi, kt * 128:(kt + 1) * 128],
                                qn[lo:hi, CH[c]], start=True, stop=True)
                        at = attnp.tile([128, N], f32r, tag="at")
                        nc.scalar.activation(out=at[:], in_=sl[:],
                                             func=AF.Exp,
                                             scale=float(D) ** -0.5)
                        for c in range(2):
                            nc.tensor.matmul(
                                av[c][:],
                                v_sb[kt][:, h * 65:h * 65 + 65],
                                at[:, CH[c]],
                                start=(kt == 0), stop=(kt == 7))
                    for c in range(2):
                        rd = rdp.tile([65, 512], f32r, tag="rd")
                        nc.vector.reciprocal(rd[64:65, :], av[c][64:65, :])
                        nbc = nbcps.tile([64, 512], f32, tag="nbc")
                        nc.tensor.matmul(nbc[:], d["ones64"][64:65, :],
                                         rd[64:65, :], start=True, stop=True)
                        nc.vector.tensor_copy(outh[h][:, CH[c]],
                                              av[c][0:64, :])
                        nc.vector.tensor_mul(outh[h][:, CH[c]],
                                             outh[h][:, CH[c]], nbc[:])
            with ExitStack() as sctx:
                yps = sctx.enter_context(
                    tc.tile_pool(name="yps", bufs=4, space="PSUM"))
                ysbp = sctx.enter_context(tc.tile_pool(name="ysb", bufs=4))
                nc.sync.dma_start(out=wps_all[:, :, 384:768],
                                  in_=wp_h[:, :, 384:768])
                for m in range(6):
                    wps = wps_all[:, :, m * 128:(m + 1) * 128]
                    for c in range(2):
                        ps = yps.tile([128, 512], f32, tag="y")
                        for k in range(12):
                            nc.tensor.matmul(ps[:], wps[:, k, :],
                                             outh[k][:, CH[c]],
                                             start=(k == 0), stop=False)
                        nc.tensor.matmul(
                            ps[:], d["wp_brow"][0:1, m * 128:(m + 1) * 128],
                            xt_ones[0:1, CH[c]], start=False, stop=True)
                        ysb = ysbp.tile([128, 512], f32, tag="ysb")
                        nc.scalar.activation(out=ysb[:], in_=ps[:],
                                             func=AF.Copy)
                        nc.sync.dma_start(
                            out=yT[m * 128:(m + 1) * 128, CH[c]], in_=ysb[:])


def _host_inputs(x, qkv_w, qkv_b, qn_w, qn_b, kn_w, kn_b, proj_w, proj_b):
    f = np.float32
    ones_row = np.ones((1, N), f)
    wqk = np.concatenate([qkv_w[:2 * C].T, qkv_b[None, :2 * C]], 0).astype(f)
    wv = np.zeros((C + 1, H * 65), f)
    for h in range(H):
        wv[0:C, h * 65:h * 65 + 64] = qkv_w[2 * C + h * 64:
                                            2 * C + (h + 1) * 64].T
        wv[C, h * 65:h * 65 + 64] = qkv_b[2 * C + h * 64:2 * C + (h + 1) * 64]
        wv[C, h * 65 + 64] = 1.0
    wp = np.concatenate([proj_w.T, proj_b[None, :]], 0).astype(f)

    def sel(sign):
        s = np.zeros((12, 12, 128), f)  # [k, m, p]
        for m in range(12):
            w_side = qn_w if m < 6 else kn_w
            ml = m % 6
            for p in range(128):
                s[2 * ml + p // 64, m, p] = sign * w_side[p % 64]
        return s

    sumsel = np.zeros((128, 6, 12), f)
    for ml in range(6):
        for p in range(128):
            sumsel[p, ml, 2 * ml + p // 64] = 1.0
    sumsel = sumsel.reshape(128, 72)
    lnb = np.stack([np.tile(qn_b, 2)[:, None],
                    np.tile(kn_b, 2)[:, None]]).astype(f)
    shared = dict(wqk=wqk, wv=wv, wp=wp, selA=sel(1.0), selE=sel(-1.0),
                  sumsel=sumsel, lnb=lnb, onesr=np.ones((1, 64), f))
    in_maps = []
    for b in range(B):
        m = dict(shared)
        m["xT"] = np.concatenate([x[b].T, ones_row], 0).astype(f)
        in_maps.append(m)
    return in_maps


def run(inputs, loop_k=1):
    from concourse.bass_utils import run_bass_kernel_spmd
    nc = _build(loop_k)
    in_maps = _host_inputs(**{k: np.asarray(v, np.float32)
                              for k, v in inputs.items()})
    res = run_bass_kernel_spmd(nc, in_maps, list(range(NCORES)))
    out = np.empty((B, N, C), np.float32)
    for b in range(B):
        out[b] = res.results[b]["yT"].T
    return out


def kernel(**inputs):
    return run(inputs, loop_k=1)
